# revision 27
# baseline (speedup 1.0000x reference)
"""Multi-head causal attention on 8 trn2 NeuronCores.

Reference semantics (B=2, S=2048, D=1024, H=16, DK=DV=64):
    q = X @ WQ * 1/sqrt(DK); k = X @ WK; v = X @ WV          (per head)
    logits[i, j] = q[i] . k[j]   (i = key pos, j = query pos, causal i <= j)
    P = softmax_i(logits); out[j] = (sum_i P[i,j] v[i]) @ WO + bO

Sharding: 2 batches x 16 heads = 32 bh-pairs -> 4 heads/core, batch b = core//4.
Each core computes attention for its heads plus the partial output projection
x_part @ WO[rows of its heads]; the host sums the 4 partials per batch
(all-reduce step of the row-sharded WO) and adds bO.

Changes vs the ACT-exp baseline (141.2us -> ~133us):
  - softmax exp is computed as the Schraudolph integer trick: pt_i16 =
    convert(logit * 2^7/ln2 + 16256), bitcast to bf16 ~= exp(logit).
    The affine+convert runs as ONE instruction on EITHER the ACT engine
    (activation Copy w/ scale+bias) or DVE (tensor_scalar), so the former
    ~79us serial ACT exp wall is split across two engines (~2/3 ACT,
    ~1/3 DVE; the last tiles go to ACT so DVE stays clear for the final
    chunks' normalize tails). The approximation bias is common-mode
    across all tiles and cancels in the softmax normalization
    (rel err 0.0093 vs 2e-2 tolerance).
  - output partials are written in bf16 (halves output DMA); the host
    combine sums them in f32.
  - each AV chunk is split into accum / tail_a (normalize + transpose)
    / tail_b (out-proj + store), pipelined 2 deep: after accum(jc) the
    weave emits tail_b(jc-2) then tail_a(jc-1), so out-proj matmuls sit
    ahead of the DVE-coupled transpose stage in the in-order PE queue.
  - xt block 0 arrives as four independent quarter tiles so the first
    projection chain starts on the first sliver.

Device schedule (globally software-pipelined, as baseline):
  score tiles are emitted from a single global cursor, deadline-driven
  before each AV chain plus pacing top-ups; diagonal masking multiplies
  run on the otherwise-idle GpSimd engine (wedge values pre-mask are
  finite garbage exps, zeroed by the 0/1 triangle).
"""

import functools

import numpy as np
import ml_dtypes

import concourse.bass as bass
import concourse.mybir as mybir
import concourse.tile as tile
from concourse import bacc
from concourse.bass_utils import run_bass_kernel_spmd
from concourse.masks import make_identity

B, S, D, H = 2, 2048, 1024, 16
DK = DV = 64
NCORES = 8
GROUP = NCORES // B          # cores per batch
HG = H // GROUP              # heads per core = 4
HD = HG * DK                 # per-core head dims = 256
P = 128
KC = D // P                  # 8 contraction chunks over D
JB = 512                     # query-block width for score matmuls
NJB = S // JB                # 4
VW = DV + 1                  # value width + ones column

PT_BUFS = 46                 # score-tile lookahead (SBUF-bounded)
NWARM = 40                   # HAM warmup matmuls

# Schraudolph constants: i16 = convert(logit * SCH_A + SCH_B); bitcast bf16
SCH_A = float(2.0**7 / np.log(2.0))
SCH_B = 16256.0

BF16 = mybir.dt.bfloat16
F32 = mybir.dt.float32
I16 = mybir.dt.int16
NPBF16 = ml_dtypes.bfloat16
Copy = mybir.ActivationFunctionType.Copy
Mult = mybir.AluOpType.mult
Add = mybir.AluOpType.add


def build_nc() -> bass.Bass:
    nc = bacc.Bacc()
    xt = nc.declare_dram_parameter("xt", [D, S], BF16, isOutput=False)
    wq = nc.declare_dram_parameter("wq", [D, HD], BF16, isOutput=False)
    wk = nc.declare_dram_parameter("wk", [D, HD], BF16, isOutput=False)
    wv = nc.declare_dram_parameter("wv", [D, HD], BF16, isOutput=False)
    wo = nc.declare_dram_parameter("wo", [HD, D], BF16, isOutput=False)
    tri = nc.declare_dram_parameter("tri", [P, P], BF16, isOutput=False)
    out = nc.declare_dram_parameter("out_part", [S, D], BF16, isOutput=True)

    out_t = out.rearrange("(c p) o -> p c o", p=P)
    xt_t = xt.rearrange("(kc p) i -> p kc i", p=P)
    wq_t = wq.rearrange("(kc p) m -> p kc m", p=P)
    wk_t = wk.rearrange("(kc p) m -> p kc m", p=P)
    wv_t = wv.rearrange("(kc p) m -> p kc m", p=P)
    wo_t = wo.rearrange("(hc p) o -> p hc o", p=P)

    with tile.TileContext(nc) as tc:
        with (
            tc.tile_pool(name="const", bufs=1) as const_pool,
            tc.tile_pool(name="big", bufs=1) as big_pool,
            tc.tile_pool(name="xth", bufs=4) as xt_pool,
            tc.tile_pool(name="pt", bufs=PT_BUFS) as pt_pool,
            tc.tile_pool(name="ptd", bufs=20) as ptd_pool,
            tc.tile_pool(name="small", bufs=3) as small_pool,
            tc.tile_pool(name="xTp", bufs=3) as xT_pool,
            tc.tile_pool(name="osb", bufs=3) as osb_pool,
            tc.tile_pool(name="mmps", bufs=2, space="PSUM") as mm_psum,
            tc.tile_pool(name="sps", bufs=2, space="PSUM") as s_psum,
            tc.tile_pool(name="avps", bufs=2, space="PSUM") as av_psum,
        ):
            # ---- constants + PE warmup (no DMA dependency) ----
            ident = const_pool.tile([P, P], BF16)
            make_identity(nc, ident)
            warm_ps = mm_psum.tile([P, JB], F32, tag="mmps")
            for _ in range(NWARM):
                nc.tensor.matmul(
                    warm_ps[:, 0:P], lhsT=ident, rhs=ident, start=True, stop=True
                )

            tri_sb = const_pool.tile([P, P], BF16)

            # ---- SBUF tiles ----
            w_sbs = {
                name: big_pool.tile([P, KC, HD], BF16, name=f"{name}_sb")
                for name in ("wq", "wk", "wv")
            }
            wo_sb = big_pool.tile([P, HD // P, D], BF16, name="wo_sb")
            # xt block halves: half (nb, lo/hi) covers kc range [4*h, 4*h+4)
            xt_halves = {}

            def load_xt_half(nb, h, eng):
                t = xt_pool.tile([P, KC // 2, JB], BF16, tag="xth")
                eng.dma_start(
                    t, xt_t[:, 4 * h : 4 * h + 4, nb * JB : (nb + 1) * JB]
                )
                xt_halves[nb, h] = t

            # block 0 arrives as four independent quarter tiles so the first
            # projection chain's kc=0 matmul starts on the first sliver
            xt_quarters = {}

            def load_xt_q0(kq, eng):
                t = xt_pool.tile([P, 2, JB], BF16, tag="xtq", name=f"xtq{kq}")
                eng.dma_start(t, xt_t[:, 2 * kq : 2 * kq + 2, 0:JB])
                xt_quarters[kq] = t

            def xt_slice(nb, kc):
                if nb == 0:
                    return xt_quarters[kc // 2][:, kc % 2, :]
                return xt_halves[nb, kc // 4][:, kc % 4, :]

            # ---- input DMA triggers, spread across idle engine queues ----
            nc.sync.dma_start(w_sbs["wk"][:, :, 0:P], wk_t[:, :, 0:P])
            for kq in range(4):
                load_xt_q0(kq, nc.sync)
            nc.scalar.dma_start(w_sbs["wq"][:, :, 0:P], wq_t[:, :, 0:P])
            nc.scalar.dma_start(w_sbs["wk"][:, :, P:HD], wk_t[:, :, P:HD])
            nc.scalar.dma_start(w_sbs["wq"][:, :, P:HD], wq_t[:, :, P:HD])
            nc.scalar.dma_start(w_sbs["wv"], wv_t)
            nc.scalar.dma_start(tri_sb, tri[:, :])
            load_xt_half(1, 0, nc.sync)
            load_xt_half(1, 1, nc.scalar)
            nc.scalar.dma_start(wo_sb, wo_t)

            qt_sb = big_pool.tile([P, HD // P, S], BF16, name="qt_sb")
            kt_sb = big_pool.tile([P, HD // P, S], BF16, name="kt_sb")
            v_sb = big_pool.tile([P, S // P, HG, VW], BF16, name="v_sb")
            nc.vector.memset(v_sb[:, :, :, DV : DV + 1], 1.0)

            # ---- projection chains ----
            def qtkt_chain(nb, w_sb, t_sb, mc):
                ps = mm_psum.tile([P, JB], F32, tag="mmps")
                for kc in range(KC):
                    nc.tensor.matmul(
                        ps,
                        lhsT=w_sb[:, kc, mc * P : (mc + 1) * P],
                        rhs=xt_slice(nb, kc),
                        start=(kc == 0),
                        stop=(kc == KC - 1),
                    )
                nc.vector.tensor_copy(t_sb[:, mc, nb * JB : (nb + 1) * JB], ps)

            def v_chain(ic):
                ps = mm_psum.tile([P, JB], F32, tag="mmps")
                for kc in range(KC):
                    nc.tensor.matmul(
                        ps[:, :HD],
                        lhsT=xt_slice(ic // 4, kc)[:, (ic % 4) * P : (ic % 4 + 1) * P],
                        rhs=w_sbs["wv"][:, kc, :],
                        start=(kc == 0),
                        stop=(kc == KC - 1),
                    )
                nc.vector.tensor_copy(
                    v_sb[:, ic, :, 0:DV],
                    ps[:, :HD].rearrange("p (h v) -> p h v", v=DV),
                )

            # ---- score tiles: global deadline-driven emission ----
            score_list = [
                (jb, ib, hp)
                for jb in range(NJB)
                for ib in range(4 * jb + 4)
                for hp in (0, 1)
            ]
            pt_tiles = {}
            ptd_tiles = {}
            qk_ready = [[False, False] for _ in range(NJB)]  # [jb][mc]
            state = {"cursor": 0, "freed": 0, "conv": 0}

            def convert(dst_bf16, src_psum):
                """pt = bitcast-bf16(i16(logit * SCH_A + SCH_B)) ~= exp."""
                c = state["conv"]
                state["conv"] += 1
                dst = dst_bf16.bitcast(I16)
                # final tiles go to ACT so DVE stays clear for the last
                # chunks' normalize/transpose tails
                if c >= len(score_list) - 12 or c % 3 != 2:
                    nc.scalar.activation(dst, src_psum, Copy, bias=SCH_B, scale=SCH_A)
                else:
                    nc.vector.tensor_scalar(dst, src_psum, SCH_A, SCH_B, Mult, Add)

            def score_tile(jb, ib, hp):
                off = max(0, (ib - 4 * jb) * P)
                sps = s_psum.tile([P, 2, JB], F32, tag="sps")
                for hh in range(2):
                    h = 2 * hp + hh
                    base = DK * (h % 2)
                    hc = h // 2
                    nc.tensor.matmul(
                        sps[:, hh, off:],
                        lhsT=qt_sb[base : base + DK, hc, ib * P : (ib + 1) * P],
                        rhs=kt_sb[base : base + DK, hc, jb * JB + off : (jb + 1) * JB],
                        start=True,
                        stop=True,
                    )
                pt = pt_pool.tile([P, 2, JB], BF16, tag="pt")
                convert(pt[:, :, off:], sps[:, :, off:])
                pt_tiles[jb, ib, hp] = pt
                if ib >= 4 * jb:  # diagonal tile: mask its jj strip on GpSimd
                    jjd = ib - 4 * jb
                    for hh in range(2):
                        ptd = ptd_pool.tile([P, P], BF16, tag="ptd")
                        nc.gpsimd.tensor_tensor(
                            ptd, pt[:, hh, jjd * P : (jjd + 1) * P], tri_sb, Mult
                        )
                        ptd_tiles[jb, ib, hp, hh] = ptd

            def tile_ready(idx):
                jb, ib, hp = score_list[idx]
                return qk_ready[jb][hp]

            def emit_scores(budget):
                while (
                    budget > 0
                    and state["cursor"] < len(score_list)
                    and tile_ready(state["cursor"])
                    and state["cursor"] - state["freed"] < PT_BUFS - 2
                ):
                    score_tile(*score_list[state["cursor"]])
                    state["cursor"] += 1
                    budget -= 1

            def force_scores(jb, jc):
                # everything this AV chain consumes must be emitted
                need = sum(
                    1
                    for (jb2, ib2, _hp) in score_list
                    if jb2 < jb or (jb2 == jb and ib2 <= jc)
                )
                while state["cursor"] < need:
                    assert tile_ready(state["cursor"]), (
                        f"score tile {score_list[state['cursor']]} not ready "
                        f"for AV chain jc={jc}"
                    )
                    score_tile(*score_list[state["cursor"]])
                    state["cursor"] += 1

            # ---- AV accumulate / (normalize + transpose + out-proj) tail ----
            # Split so the weave can queue the NEXT chunk's AV matmuls on the
            # in-order PE stream before this chunk's tail (whose transposes
            # and out-proj wait on DVE normalize work) — keeps the PE fed.
            av_combs = {}

            def av_accum(jb, jj):
                jc = 4 * jb + jj
                comb = av_psum.tile([P, 448], F32, tag="avps")
                av_combs[jc] = comb
                xps = comb[:, 0 : HG * VW].rearrange("p (h v) -> p h v", v=VW)
                for h in range(HG):
                    hp, hh = divmod(h, 2)
                    for ib in range(jc + 1):
                        if ib == jc:
                            lhsT = ptd_tiles[jb, ib, hp, hh]
                        else:
                            lhsT = pt_tiles[jb, ib, hp][:, hh, jj * P : (jj + 1) * P]
                        nc.tensor.matmul(
                            xps[:, h, :],
                            lhsT=lhsT,
                            rhs=v_sb[:, ib, h, :],
                            start=(ib == 0),
                            stop=(ib == jc),
                        )

            av_xts = {}

            def av_tail_a(jc):
                # normalize + transpose + xT eviction (DVE-coupled stage)
                comb = av_combs.pop(jc)
                xps = comb[:, 0 : HG * VW].rearrange("p (h v) -> p h v", v=VW)
                recip = small_pool.tile([P, HG], F32, tag="recip")
                nc.vector.reciprocal(recip, xps[:, :, DV])
                xn = small_pool.tile([P, HG, DV], BF16, tag="xn")
                nc.vector.tensor_tensor(
                    xn,
                    xps[:, :, 0:DV],
                    recip[:, :, None].to_broadcast([P, HG, DV]),
                    Mult,
                )
                xn_flat = xn.rearrange("p h v -> p (h v)")
                xT = xT_pool.tile([P, HD // P, P], BF16, tag="xT")
                for vc in range(HD // P):
                    tps = comb[:, 288 + vc * DV : 288 + (vc + 1) * DV].bitcast(BF16)
                    nc.tensor.transpose(tps, xn_flat[:, vc * P : (vc + 1) * P], ident)
                    nc.vector.tensor_copy(xT[:, vc, :], tps)
                av_xts[jc] = xT

            def av_tail_b(jc):
                # output projection + store (pure PE + copy stage)
                xT = av_xts.pop(jc)
                for oc in range(D // JB):
                    ops = mm_psum.tile([P, JB], F32, tag="mmps")
                    for hc in range(HD // P):
                        nc.tensor.matmul(
                            ops,
                            lhsT=xT[:, hc, :],
                            rhs=wo_sb[:, hc, oc * JB : (oc + 1) * JB],
                            start=(hc == 0),
                            stop=(hc == HD // P - 1),
                        )
                    osb = osb_pool.tile([P, JB], BF16, tag="osb")
                    if oc == 0:
                        nc.scalar.copy(osb, ops)
                    else:
                        nc.vector.tensor_copy(osb, ops)
                    nc.sync.dma_start(out_t[:, jc, oc * JB : (oc + 1) * JB], osb)

            # ---- PRE: projection block 0, first score tiles ----
            qtkt_chain(0, w_sbs["wk"], kt_sb, 0)
            qtkt_chain(0, w_sbs["wq"], qt_sb, 0)
            qk_ready[0][0] = True
            qtkt_chain(0, w_sbs["wk"], kt_sb, 1)
            qtkt_chain(0, w_sbs["wq"], qt_sb, 1)
            qk_ready[0][1] = True
            emit_scores(2)
            v_chain(0)
            emit_scores(2)
            v_chain(1)
            emit_scores(2)
            v_chain(2)
            emit_scores(2)
            v_chain(3)

            # ---- main weave ----
            TOPUP = {0: 3, 1: 4, 2: 4, 3: 2}
            for jb in range(NJB):
                for jj in range(4):
                    jc = 4 * jb + jj
                    nb = jb + 1
                    if jb == 0 and jj == 0 and NJB > 2:
                        load_xt_half(2, 0, nc.sync)
                        load_xt_half(2, 1, nc.sync)
                    if jb == 1 and jj == 0 and NJB > 3:
                        load_xt_half(3, 0, nc.sync)
                        load_xt_half(3, 1, nc.sync)
                    force_scores(jb, jc)
                    emit_scores(TOPUP[jb])
                    av_accum(jb, jj)
                    if jc > 1:
                        av_tail_b(jc - 2)
                    # next-block projection chains run AFTER the deadline path:
                    # they are pure PE filler during this chunk's DVE tail and
                    # aren't consumed for another ~4 chunks
                    if nb < NJB:
                        if jj == 0:
                            qtkt_chain(nb, w_sbs["wk"], kt_sb, 0)
                            qtkt_chain(nb, w_sbs["wq"], qt_sb, 0)
                            qk_ready[nb][0] = True
                        elif jj == 1:
                            qtkt_chain(nb, w_sbs["wk"], kt_sb, 1)
                            qtkt_chain(nb, w_sbs["wq"], qt_sb, 1)
                            qk_ready[nb][1] = True
                        elif jj == 2:
                            v_chain(4 * nb + 0)
                            v_chain(4 * nb + 1)
                        else:
                            v_chain(4 * nb + 2)
                            v_chain(4 * nb + 3)
                    emit_scores(3)
                    if jc > 0:
                        av_tail_a(jc - 1)
                state["freed"] += 2 * (4 * jb + 4)
            av_tail_b(S // P - 2)
            av_tail_a(S // P - 1)
            av_tail_b(S // P - 1)
            assert state["cursor"] == len(score_list)
    nc.compile()
    return nc


@functools.lru_cache(maxsize=1)
def _cached_nc() -> bass.Bass:
    return build_nc()


def make_in_maps(inputs, mask, WQ, WK, WV, WO, bO):
    scale = np.float32(1.0 / np.sqrt(DK))
    wq2 = np.ascontiguousarray((WQ.reshape(D, D) * scale).astype(NPBF16))
    wk2 = np.ascontiguousarray(WK.reshape(D, D).astype(NPBF16))
    wv2 = np.ascontiguousarray(WV.reshape(D, D).astype(NPBF16))
    wo2 = np.ascontiguousarray(WO.astype(NPBF16))
    tri = np.triu(np.ones((P, P), np.float32)).astype(NPBF16)
    xts = [
        np.ascontiguousarray(np.asarray(inputs[b]).T.astype(NPBF16)) for b in range(B)
    ]
    in_maps = []
    for c in range(NCORES):
        b, hg = divmod(c, GROUP)
        cols = slice(hg * HD, (hg + 1) * HD)
        in_maps.append(
            {
                "xt": xts[b],
                "wq": np.ascontiguousarray(wq2[:, cols]),
                "wk": np.ascontiguousarray(wk2[:, cols]),
                "wv": np.ascontiguousarray(wv2[:, cols]),
                "wo": np.ascontiguousarray(wo2[cols, :]),
                "tri": tri,
            }
        )
    return in_maps


def combine(results, bO):
    parts = [r["out_part"] for r in results]
    out = np.empty((B, S, D), np.float32)
    for b in range(B):
        acc = parts[b * GROUP].astype(np.float32)
        for g in range(1, GROUP):
            acc = acc + parts[b * GROUP + g].astype(np.float32)
        out[b] = acc + np.asarray(bO, np.float32)[None, :]
    return out


def kernel(**inputs) -> np.ndarray:
    nc = _cached_nc()
    in_maps = make_in_maps(**inputs)
    res = run_bass_kernel_spmd(nc, in_maps, core_ids=list(range(NCORES)))
    return combine(res.results, inputs["bO"])


# revision 29
# speedup vs baseline: 1.0231x; 1.0231x over previous
"""Multi-head causal attention on 8 trn2 NeuronCores.

Reference semantics (B=2, S=2048, D=1024, H=16, DK=DV=64):
    q = X @ WQ * 1/sqrt(DK); k = X @ WK; v = X @ WV          (per head)
    logits[i, j] = q[i] . k[j]   (i = key pos, j = query pos, causal i <= j)
    P = softmax_i(logits); out[j] = (sum_i P[i,j] v[i]) @ WO + bO

Sharding: 2 batches x 16 heads = 32 bh-pairs -> 4 heads/core, batch b = core//4.
Each core computes attention for its heads plus the partial output projection
x_part @ WO[rows of its heads]; the host sums the 4 partials per batch
(all-reduce step of the row-sharded WO) and adds bO.

Changes vs the ACT-exp baseline (141.2us -> ~133us):
  - softmax exp is computed as the Schraudolph integer trick: pt_i16 =
    convert(logit * 2^7/ln2 + 16256), bitcast to bf16 ~= exp(logit).
    The affine+convert runs as ONE instruction on EITHER the ACT engine
    (activation Copy w/ scale+bias) or DVE (tensor_scalar), so the former
    ~79us serial ACT exp wall is split across two engines (~2/3 ACT,
    ~1/3 DVE; the last tiles go to ACT so DVE stays clear for the final
    chunks' normalize tails). The approximation bias is common-mode
    across all tiles and cancels in the softmax normalization
    (rel err 0.0093 vs 2e-2 tolerance).
  - output partials are written in bf16 (halves output DMA); the host
    combine sums them in f32.
  - each AV chunk is split into accum / tail_a (normalize + transpose)
    / tail_b (out-proj + store), pipelined 2 deep: after accum(jc) the
    weave emits tail_b(jc-2) then tail_a(jc-1), so out-proj matmuls sit
    ahead of the DVE-coupled transpose stage in the in-order PE queue.
  - xt block 0 arrives as four independent quarter tiles so the first
    projection chain starts on the first sliver.

Device schedule (globally software-pipelined, as baseline):
  score tiles are emitted from a single global cursor, deadline-driven
  before each AV chain plus pacing top-ups; diagonal masking multiplies
  run on the otherwise-idle GpSimd engine (wedge values pre-mask are
  finite garbage exps, zeroed by the 0/1 triangle).
"""

import functools

import numpy as np
import ml_dtypes

import concourse.bass as bass
import concourse.mybir as mybir
import concourse.tile as tile
from concourse import bacc
from concourse.bass_utils import run_bass_kernel_spmd
from concourse.masks import make_identity

B, S, D, H = 2, 2048, 1024, 16
DK = DV = 64
NCORES = 8
GROUP = NCORES // B          # cores per batch
HG = H // GROUP              # heads per core = 4
HD = HG * DK                 # per-core head dims = 256
P = 128
KC = D // P                  # 8 contraction chunks over D
JB = 512                     # query-block width for score matmuls
NJB = S // JB                # 4
VW = DV + 1                  # value width + ones column

PT_BUFS = 46                 # score-tile lookahead (SBUF-bounded)
NWARM = 40                   # HAM warmup matmuls

# Schraudolph constants: i16 = convert(logit * SCH_A + SCH_B); bitcast bf16
SCH_A = float(2.0**7 / np.log(2.0))
SCH_B = 16256.0

BF16 = mybir.dt.bfloat16
F32 = mybir.dt.float32
I16 = mybir.dt.int16
NPBF16 = ml_dtypes.bfloat16
Copy = mybir.ActivationFunctionType.Copy
Mult = mybir.AluOpType.mult
Add = mybir.AluOpType.add


def build_nc() -> bass.Bass:
    nc = bacc.Bacc()
    xt = nc.declare_dram_parameter("xt", [D, S], BF16, isOutput=False)
    wq = nc.declare_dram_parameter("wq", [D, HD], BF16, isOutput=False)
    wk = nc.declare_dram_parameter("wk", [D, HD], BF16, isOutput=False)
    wv = nc.declare_dram_parameter("wv", [D, HD], BF16, isOutput=False)
    wo = nc.declare_dram_parameter("wo", [HD, D], BF16, isOutput=False)
    tri = nc.declare_dram_parameter("tri", [P, P], BF16, isOutput=False)
    out = nc.declare_dram_parameter("out_part", [S, D], BF16, isOutput=True)

    out_t = out.rearrange("(c p) o -> p c o", p=P)
    xt_t = xt.rearrange("(kc p) i -> p kc i", p=P)
    wq_t = wq.rearrange("(kc p) m -> p kc m", p=P)
    wk_t = wk.rearrange("(kc p) m -> p kc m", p=P)
    wv_t = wv.rearrange("(kc p) m -> p kc m", p=P)
    wo_t = wo.rearrange("(hc p) o -> p hc o", p=P)

    with tile.TileContext(nc) as tc:
        with (
            tc.tile_pool(name="const", bufs=1) as const_pool,
            tc.tile_pool(name="big", bufs=1) as big_pool,
            tc.tile_pool(name="xth", bufs=4) as xt_pool,
            tc.tile_pool(name="pt", bufs=PT_BUFS) as pt_pool,
            tc.tile_pool(name="ptd", bufs=20) as ptd_pool,
            tc.tile_pool(name="small", bufs=3) as small_pool,
            tc.tile_pool(name="xTp", bufs=3) as xT_pool,
            tc.tile_pool(name="osb", bufs=3) as osb_pool,
            tc.tile_pool(name="mmps", bufs=2, space="PSUM") as mm_psum,
            tc.tile_pool(name="sps", bufs=2, space="PSUM") as s_psum,
            tc.tile_pool(name="avps", bufs=2, space="PSUM") as av_psum,
        ):
            # ---- constants + PE warmup (no DMA dependency) ----
            ident = const_pool.tile([P, P], BF16)
            make_identity(nc, ident)
            warm_ps = mm_psum.tile([P, JB], F32, tag="mmps")
            for _ in range(NWARM):
                nc.tensor.matmul(
                    warm_ps[:, 0:P], lhsT=ident, rhs=ident, start=True, stop=True
                )

            tri_sb = const_pool.tile([P, P], BF16)

            # ---- SBUF tiles ----
            w_sbs = {
                name: big_pool.tile([P, KC, HD], BF16, name=f"{name}_sb")
                for name in ("wq", "wk", "wv")
            }
            wo_sb = big_pool.tile([P, HD // P, D], BF16, name="wo_sb")
            # xt block halves: half (nb, lo/hi) covers kc range [4*h, 4*h+4)
            xt_halves = {}

            def load_xt_half(nb, h, eng):
                t = xt_pool.tile([P, KC // 2, JB], BF16, tag="xth")
                eng.dma_start(
                    t, xt_t[:, 4 * h : 4 * h + 4, nb * JB : (nb + 1) * JB]
                )
                xt_halves[nb, h] = t

            # block 0 arrives as four independent quarter tiles so the first
            # projection chain's kc=0 matmul starts on the first sliver
            xt_quarters = {}

            def load_xt_q0(kq, eng):
                t = xt_pool.tile([P, 2, JB], BF16, tag="xtq", name=f"xtq{kq}")
                eng.dma_start(t, xt_t[:, 2 * kq : 2 * kq + 2, 0:JB])
                xt_quarters[kq] = t

            def xt_slice(nb, kc):
                if nb == 0:
                    return xt_quarters[kc // 2][:, kc % 2, :]
                return xt_halves[nb, kc // 4][:, kc % 4, :]

            # ---- input DMA triggers, spread across idle engine queues ----
            nc.sync.dma_start(w_sbs["wk"][:, :, 0:P], wk_t[:, :, 0:P])
            for kq in range(4):
                load_xt_q0(kq, nc.sync)
            nc.scalar.dma_start(w_sbs["wq"][:, :, 0:P], wq_t[:, :, 0:P])
            nc.scalar.dma_start(w_sbs["wk"][:, :, P:HD], wk_t[:, :, P:HD])
            nc.scalar.dma_start(w_sbs["wq"][:, :, P:HD], wq_t[:, :, P:HD])
            nc.scalar.dma_start(w_sbs["wv"], wv_t)
            nc.scalar.dma_start(tri_sb, tri[:, :])
            load_xt_half(1, 0, nc.sync)
            load_xt_half(1, 1, nc.scalar)
            nc.scalar.dma_start(wo_sb, wo_t)

            qt_sb = big_pool.tile([P, HD // P, S], BF16, name="qt_sb")
            kt_sb = big_pool.tile([P, HD // P, S], BF16, name="kt_sb")
            v_sb = big_pool.tile([P, S // P, HG, VW], BF16, name="v_sb")
            nc.vector.memset(v_sb[:, :, :, DV : DV + 1], 1.0)

            # ---- projection chains ----
            def qtkt_chain(nb, w_sb, t_sb, mc):
                ps = mm_psum.tile([P, JB], F32, tag="mmps")
                for kc in range(KC):
                    nc.tensor.matmul(
                        ps,
                        lhsT=w_sb[:, kc, mc * P : (mc + 1) * P],
                        rhs=xt_slice(nb, kc),
                        start=(kc == 0),
                        stop=(kc == KC - 1),
                    )
                nc.vector.tensor_copy(t_sb[:, mc, nb * JB : (nb + 1) * JB], ps)

            def v_chain(ic):
                ps = mm_psum.tile([P, JB], F32, tag="mmps")
                for kc in range(KC):
                    nc.tensor.matmul(
                        ps[:, :HD],
                        lhsT=xt_slice(ic // 4, kc)[:, (ic % 4) * P : (ic % 4 + 1) * P],
                        rhs=w_sbs["wv"][:, kc, :],
                        start=(kc == 0),
                        stop=(kc == KC - 1),
                    )
                nc.vector.tensor_copy(
                    v_sb[:, ic, :, 0:DV],
                    ps[:, :HD].rearrange("p (h v) -> p h v", v=DV),
                )

            # ---- score tiles: global deadline-driven emission ----
            score_list = [
                (jb, ib, hp)
                for jb in range(NJB)
                for ib in range(4 * jb + 4)
                for hp in (0, 1)
            ]
            pt_tiles = {}
            ptd_tiles = {}
            qk_ready = [[False, False] for _ in range(NJB)]  # [jb][mc]
            state = {"cursor": 0, "freed": 0, "conv": 0}

            def convert(dst_bf16, src_psum):
                """pt = bitcast-bf16(i16(logit * SCH_A + SCH_B)) ~= exp."""
                c = state["conv"]
                state["conv"] += 1
                dst = dst_bf16.bitcast(I16)
                # final tiles go to ACT so DVE stays clear for the last
                # chunks' normalize/transpose tails
                if c >= len(score_list) - 12 or c % 4 != 3:
                    nc.scalar.activation(dst, src_psum, Copy, bias=SCH_B, scale=SCH_A)
                else:
                    nc.vector.tensor_scalar(dst, src_psum, SCH_A, SCH_B, Mult, Add)

            def score_tile(jb, ib, hp):
                off = max(0, (ib - 4 * jb) * P)
                sps = s_psum.tile([P, 2, JB], F32, tag="sps")
                for hh in range(2):
                    h = 2 * hp + hh
                    base = DK * (h % 2)
                    hc = h // 2
                    nc.tensor.matmul(
                        sps[:, hh, off:],
                        lhsT=qt_sb[base : base + DK, hc, ib * P : (ib + 1) * P],
                        rhs=kt_sb[base : base + DK, hc, jb * JB + off : (jb + 1) * JB],
                        start=True,
                        stop=True,
                    )
                pt = pt_pool.tile([P, 2, JB], BF16, tag="pt")
                convert(pt[:, :, off:], sps[:, :, off:])
                pt_tiles[jb, ib, hp] = pt
                if ib >= 4 * jb:  # diagonal tile: mask its jj strip on GpSimd
                    jjd = ib - 4 * jb
                    for hh in range(2):
                        ptd = ptd_pool.tile([P, P], BF16, tag="ptd")
                        nc.gpsimd.tensor_tensor(
                            ptd, pt[:, hh, jjd * P : (jjd + 1) * P], tri_sb, Mult
                        )
                        ptd_tiles[jb, ib, hp, hh] = ptd

            def tile_ready(idx):
                jb, ib, hp = score_list[idx]
                return qk_ready[jb][hp]

            def emit_scores(budget):
                while (
                    budget > 0
                    and state["cursor"] < len(score_list)
                    and tile_ready(state["cursor"])
                    and state["cursor"] - state["freed"] < PT_BUFS - 2
                ):
                    score_tile(*score_list[state["cursor"]])
                    state["cursor"] += 1
                    budget -= 1

            def force_scores(jb, jc):
                # everything this AV chain consumes must be emitted
                need = sum(
                    1
                    for (jb2, ib2, _hp) in score_list
                    if jb2 < jb or (jb2 == jb and ib2 <= jc)
                )
                while state["cursor"] < need:
                    assert tile_ready(state["cursor"]), (
                        f"score tile {score_list[state['cursor']]} not ready "
                        f"for AV chain jc={jc}"
                    )
                    score_tile(*score_list[state["cursor"]])
                    state["cursor"] += 1

            # ---- AV accumulate / (normalize + transpose + out-proj) tail ----
            # Split so the weave can queue the NEXT chunk's AV matmuls on the
            # in-order PE stream before this chunk's tail (whose transposes
            # and out-proj wait on DVE normalize work) — keeps the PE fed.
            av_combs = {}

            def av_accum(jb, jj):
                jc = 4 * jb + jj
                comb = av_psum.tile([P, 448], F32, tag="avps")
                av_combs[jc] = comb
                xps = comb[:, 0 : HG * VW].rearrange("p (h v) -> p h v", v=VW)
                for h in range(HG):
                    hp, hh = divmod(h, 2)
                    for ib in range(jc + 1):
                        if ib == jc:
                            lhsT = ptd_tiles[jb, ib, hp, hh]
                        else:
                            lhsT = pt_tiles[jb, ib, hp][:, hh, jj * P : (jj + 1) * P]
                        nc.tensor.matmul(
                            xps[:, h, :],
                            lhsT=lhsT,
                            rhs=v_sb[:, ib, h, :],
                            start=(ib == 0),
                            stop=(ib == jc),
                        )

            av_xts = {}

            def av_tail_a(jc):
                # normalize + transpose + xT eviction (DVE-coupled stage)
                comb = av_combs.pop(jc)
                xps = comb[:, 0 : HG * VW].rearrange("p (h v) -> p h v", v=VW)
                recip = small_pool.tile([P, HG], F32, tag="recip")
                nc.vector.reciprocal(recip, xps[:, :, DV])
                xn = small_pool.tile([P, HG, DV], BF16, tag="xn")
                nc.vector.tensor_tensor(
                    xn,
                    xps[:, :, 0:DV],
                    recip[:, :, None].to_broadcast([P, HG, DV]),
                    Mult,
                )
                xn_flat = xn.rearrange("p h v -> p (h v)")
                xT = xT_pool.tile([P, HD // P, P], BF16, tag="xT")
                for vc in range(HD // P):
                    tps = comb[:, 288 + vc * DV : 288 + (vc + 1) * DV].bitcast(BF16)
                    nc.tensor.transpose(tps, xn_flat[:, vc * P : (vc + 1) * P], ident)
                    nc.vector.tensor_copy(xT[:, vc, :], tps)
                av_xts[jc] = xT

            def av_tail_b(jc):
                # output projection + store (pure PE + copy stage)
                xT = av_xts.pop(jc)
                for oc in range(D // JB):
                    ops = mm_psum.tile([P, JB], F32, tag="mmps")
                    for hc in range(HD // P):
                        nc.tensor.matmul(
                            ops,
                            lhsT=xT[:, hc, :],
                            rhs=wo_sb[:, hc, oc * JB : (oc + 1) * JB],
                            start=(hc == 0),
                            stop=(hc == HD // P - 1),
                        )
                    osb = osb_pool.tile([P, JB], BF16, tag="osb")
                    if oc == 0:
                        nc.scalar.copy(osb, ops)
                    else:
                        nc.vector.tensor_copy(osb, ops)
                    nc.sync.dma_start(out_t[:, jc, oc * JB : (oc + 1) * JB], osb)

            # ---- PRE: projection block 0, first score tiles ----
            qtkt_chain(0, w_sbs["wk"], kt_sb, 0)
            qtkt_chain(0, w_sbs["wq"], qt_sb, 0)
            qk_ready[0][0] = True
            qtkt_chain(0, w_sbs["wk"], kt_sb, 1)
            qtkt_chain(0, w_sbs["wq"], qt_sb, 1)
            qk_ready[0][1] = True
            emit_scores(2)
            v_chain(0)
            emit_scores(2)
            v_chain(1)
            emit_scores(2)
            v_chain(2)
            emit_scores(2)
            v_chain(3)

            # ---- main weave ----
            TOPUP = {0: 3, 1: 4, 2: 4, 3: 2}
            for jb in range(NJB):
                for jj in range(4):
                    jc = 4 * jb + jj
                    nb = jb + 1
                    if nb < NJB:
                        if jj == 0:
                            qtkt_chain(nb, w_sbs["wk"], kt_sb, 0)
                            qtkt_chain(nb, w_sbs["wq"], qt_sb, 0)
                            qk_ready[nb][0] = True
                        elif jj == 1:
                            qtkt_chain(nb, w_sbs["wk"], kt_sb, 1)
                            qtkt_chain(nb, w_sbs["wq"], qt_sb, 1)
                            qk_ready[nb][1] = True
                        elif jj == 2:
                            v_chain(4 * nb + 0)
                            v_chain(4 * nb + 1)
                        else:
                            v_chain(4 * nb + 2)
                            v_chain(4 * nb + 3)
                    if jb == 0 and jj == 0 and NJB > 2:
                        load_xt_half(2, 0, nc.sync)
                        load_xt_half(2, 1, nc.sync)
                    if jb == 1 and jj == 0 and NJB > 3:
                        load_xt_half(3, 0, nc.sync)
                        load_xt_half(3, 1, nc.sync)
                    force_scores(jb, jc)
                    emit_scores(TOPUP[jb])
                    av_accum(jb, jj)
                    if jc > 1:
                        av_tail_b(jc - 2)
                    emit_scores(3)
                    if jc > 0:
                        av_tail_a(jc - 1)
                state["freed"] += 2 * (4 * jb + 4)
            av_tail_b(S // P - 2)
            av_tail_a(S // P - 1)
            av_tail_b(S // P - 1)
            assert state["cursor"] == len(score_list)
    nc.compile()
    return nc


@functools.lru_cache(maxsize=1)
def _cached_nc() -> bass.Bass:
    return build_nc()


def make_in_maps(inputs, mask, WQ, WK, WV, WO, bO):
    scale = np.float32(1.0 / np.sqrt(DK))
    wq2 = np.ascontiguousarray((WQ.reshape(D, D) * scale).astype(NPBF16))
    wk2 = np.ascontiguousarray(WK.reshape(D, D).astype(NPBF16))
    wv2 = np.ascontiguousarray(WV.reshape(D, D).astype(NPBF16))
    wo2 = np.ascontiguousarray(WO.astype(NPBF16))
    tri = np.triu(np.ones((P, P), np.float32)).astype(NPBF16)
    xts = [
        np.ascontiguousarray(np.asarray(inputs[b]).T.astype(NPBF16)) for b in range(B)
    ]
    in_maps = []
    for c in range(NCORES):
        b, hg = divmod(c, GROUP)
        cols = slice(hg * HD, (hg + 1) * HD)
        in_maps.append(
            {
                "xt": xts[b],
                "wq": np.ascontiguousarray(wq2[:, cols]),
                "wk": np.ascontiguousarray(wk2[:, cols]),
                "wv": np.ascontiguousarray(wv2[:, cols]),
                "wo": np.ascontiguousarray(wo2[cols, :]),
                "tri": tri,
            }
        )
    return in_maps


def combine(results, bO):
    parts = [r["out_part"] for r in results]
    out = np.empty((B, S, D), np.float32)
    for b in range(B):
        acc = parts[b * GROUP].astype(np.float32)
        for g in range(1, GROUP):
            acc = acc + parts[b * GROUP + g].astype(np.float32)
        out[b] = acc + np.asarray(bO, np.float32)[None, :]
    return out


def kernel(**inputs) -> np.ndarray:
    nc = _cached_nc()
    in_maps = make_in_maps(**inputs)
    res = run_bass_kernel_spmd(nc, in_maps, core_ids=list(range(NCORES)))
    return combine(res.results, inputs["bO"])
